# revision 19
# baseline (speedup 1.0000x reference)
"""Trainium2 Bass kernel for nn_ConstraintsModule.

Reference math:
    m = preds[:, atoms]                                   # [B, N]
    body_rev[b,c,j] = pos_body[c,j] + m[b,j]*(neg_body-pos_body)[c,j]
    body_min[b,c]   = 1 - max_j body_rev[b,c,j]
    lb[b,n] = max_c body_min[b,c]*pos_head[c,n]
    ub[b,n] = 1 - max_c body_min[b,c]*neg_head[c,n]
    updated = clamp(m, min(lb,ub), max(lb,ub))
    out = preds with columns `atoms` replaced by updated

Structure exploited:
  * body masks are ~2% dense: max_j body_rev collapses to
    max(1 - min_{j in pos} m, max_{j in neg} m) over ~20 literals.
  * head rows are one-hot: lb/ub are segment maxes of body_min grouped by
    (head atom, sign).

Layout: all 128 batch rows on the SBUF partition axis.  The host packs,
per constraint slot, pos-literal m values (padded with 1.0) and
neg-literal m values (padded with 0.0); slots are grouped into a "light"
region (small uniform width) and a "heavy" region (full width) to cut
padding bytes.  Device work is pure DVE: strided tensor_reduce min/max
per region chunk (overlapped with the chunk DMAs), an exact
body_min = 1-max(1-minP, maxQ) rewrite, per-(atom-group) segment maxes,
and the final clamp.  Every op rounds exactly like the reference, so the
result is bit-identical to the fp32 reference.

Sharding: atoms are grouped by (heavy, pos-bin-size, neg-bin-size) and
dealt round-robin to the 8 cores, so all cores share one SPMD program
(groups padded to the cross-core max count); only packed data differs.
"""

import sys
from contextlib import ExitStack

import numpy as np

if "/opt/trn_rl_repo" not in sys.path:
    sys.path.insert(0, "/opt/trn_rl_repo")

import concourse.bacc as bacc
import concourse.tile as tile
from concourse import mybir
from concourse.bass_utils import run_bass_kernel_spmd

B = 128
C = 1024
N = 512
NCORES = 8
N_LIGHT_CHUNKS = 5

# Set by test.py to profile; the grading path leaves these alone.
_TRACE = False
_LAST_RESULTS = None

_PROGRAM_CACHE: dict = {}


def _roundup(x: int, mult: int) -> int:
    return ((x + mult - 1) // mult) * mult


def _chunk_plan(kpl, knl, kph, knh, sl_pad, sh_pad):
    """Graded chunks (small first, so the first DVE reduce can start as
    early as possible) over [light slots][heavy slots], alternating between
    the two fast HWDGE queues (scalar, gpsimd)."""
    wl, wh = kpl + knl, kph + knh
    work = [("l", sl_pad, wl)]
    if sh_pad:
        work.append(("h", sh_pad, wh))
    total = sl_pad * wl + sh_pad * wh
    fracs = [0.05, 0.05, 0.09, 0.09, 0.145, 0.145, 0.215, 0.21]
    chunks = []  # (region, s0, s1)
    wi, s = 0, 0
    for f in fracs:
        budget = max(int(total * f), 1)
        while budget > 0 and wi < len(work):
            reg, nslots, w = work[wi]
            take = min(max(budget // w, 1), nslots - s)
            if take > 0:
                chunks.append((reg, s, s + take))
                budget -= take * w
                s += take
            if s >= nslots:
                wi += 1
                s = 0
    while wi < len(work):
        reg, nslots, w = work[wi]
        if s < nslots:
            chunks.append((reg, s, nslots))
        wi += 1
        s = 0
    # merge adjacent same-region chunks created by boundary spill
    merged = []
    for c in chunks:
        if merged and merged[-1][0] == c[0] and merged[-1][2] == c[1]:
            last = merged.pop()
            merged.append((c[0], last[1], c[2]))
        else:
            merged.append(c)
    return tuple(merged)


def _build_program(dims, groups):
    """dims = (kpl, knl, kph, knh, sl_pad, sh_pad, nl_pad);
    groups: tuple of (sp, sn, cnt, col_off, slot_off) in the combined slot
    space (light slots first, then heavy)."""
    key = (dims, groups)
    if key in _PROGRAM_CACHE:
        return _PROGRAM_CACHE[key]
    kpl, knl, kph, knh, sl_pad, sh_pad, nl_pad = dims

    dt = mybir.dt
    wl, wh = kpl + knl, kph + knh
    s_tot = sl_pad + sh_pad
    chunks = _chunk_plan(kpl, knl, kph, knh, sl_pad, sh_pad)

    nc = bacc.Bacc(
        "TRN2", target_bir_lowering=False, debug=False, enable_partition_id=False
    )
    c_ds = [
        nc.dram_tensor(
            f"c{i}", [B, (s1 - s0) * (wl if reg == "l" else wh)], dt.float32,
            kind="ExternalInput",
        )
        for i, (reg, s0, s1) in enumerate(chunks)
    ]
    mloc_d = nc.dram_tensor("mloc", [B, nl_pad], dt.float32, kind="ExternalInput")
    out_d = nc.dram_tensor("upd", [B, nl_pad], dt.float32, kind="ExternalOutput")

    with ExitStack() as ctx:
        tc = ctx.enter_context(tile.TileContext(nc))
        pool = ctx.enter_context(tc.tile_pool(name="main", bufs=1))

        mloc_sb = pool.tile([B, nl_pad], dt.float32, tag="mloc")
        nc.sync.dma_start(mloc_sb[:], mloc_d.ap())

        gl_sb = pool.tile([B, sl_pad * wl], dt.float32, tag="gl")
        gh_sb = pool.tile([B, max(sh_pad, 1) * wh], dt.float32, tag="gh")
        minp_sb = pool.tile([B, s_tot], dt.float32, tag="minp")
        maxq_sb = pool.tile([B, s_tot], dt.float32, tag="maxq")
        # G rides only the two fast HWDGE queues; sync carries mloc/out.
        dma_engines = [nc.scalar, nc.gpsimd]
        for i, (reg, s0, s1) in enumerate(chunks):
            w, kp_w, g_t, base = (
                (wl, kpl, gl_sb, 0) if reg == "l" else (wh, kph, gh_sb, sl_pad)
            )
            dma_engines[i % 2].dma_start(g_t[:, s0 * w : s1 * w], c_ds[i].ap())
            g3 = g_t[:, s0 * w : s1 * w].rearrange("p (c k) -> p c k", k=w)
            nc.vector.tensor_reduce(
                minp_sb[:, base + s0 : base + s1], g3[:, :, 0:kp_w],
                axis=mybir.AxisListType.X, op=mybir.AluOpType.min,
            )
            nc.vector.tensor_reduce(
                maxq_sb[:, base + s0 : base + s1], g3[:, :, kp_w:w],
                axis=mybir.AxisListType.X, op=mybir.AluOpType.max,
            )

        # body_min = 1 - max(1 - minP, maxQ), rounded exactly like the
        # reference (which materializes each 1-m and 1-body_max).
        bmin_sb = pool.tile([B, s_tot], dt.float32, tag="bmin")
        nc.vector.tensor_scalar(
            minp_sb[:], minp_sb[:], -1.0, 1.0,
            op0=mybir.AluOpType.mult, op1=mybir.AluOpType.add,
        )
        nc.vector.tensor_tensor(
            minp_sb[:], minp_sb[:], maxq_sb[:], op=mybir.AluOpType.max
        )
        nc.vector.tensor_scalar(
            bmin_sb[:], minp_sb[:], -1.0, 1.0,
            op0=mybir.AluOpType.mult, op1=mybir.AluOpType.add,
        )

        # Head phase: segment maxes over (atom, sign) bins.
        lb_sb = pool.tile([B, nl_pad], dt.float32, tag="lb")
        ubm_sb = pool.tile([B, nl_pad], dt.float32, tag="ubm")
        nc.vector.memset(lb_sb[:], 0.0)
        nc.vector.memset(ubm_sb[:], 0.0)
        for sp, sn, cnt, col_off, slot_off in groups:
            w = sp + sn
            if w == 0:
                continue  # lb/ubm stay 0 from the memset
            seg = bmin_sb[:, slot_off : slot_off + cnt * w].rearrange(
                "p (a l) -> p a l", l=w
            )
            if sp > 0:
                nc.vector.tensor_reduce(
                    lb_sb[:, col_off : col_off + cnt], seg[:, :, 0:sp],
                    axis=mybir.AxisListType.X, op=mybir.AluOpType.max,
                )
            if sn > 0:
                nc.vector.tensor_reduce(
                    ubm_sb[:, col_off : col_off + cnt], seg[:, :, sp:w],
                    axis=mybir.AxisListType.X, op=mybir.AluOpType.max,
                )

        # updated = max(min(lb, ub), min(max(lb, ub), m)),  ub = 1 - ubm
        ub_sb = pool.tile([B, nl_pad], dt.float32, tag="ub")
        nc.vector.tensor_scalar(
            ub_sb[:], ubm_sb[:], -1.0, 1.0,
            op0=mybir.AluOpType.mult, op1=mybir.AluOpType.add,
        )
        lo_sb = pool.tile([B, nl_pad], dt.float32, tag="lo")
        nc.vector.tensor_tensor(lo_sb[:], lb_sb[:], ub_sb[:], op=mybir.AluOpType.min)
        hi_sb = pool.tile([B, nl_pad], dt.float32, tag="hi")
        nc.vector.tensor_tensor(hi_sb[:], lb_sb[:], ub_sb[:], op=mybir.AluOpType.max)
        upd_sb = pool.tile([B, nl_pad], dt.float32, tag="upd")
        nc.vector.tensor_tensor(upd_sb[:], hi_sb[:], mloc_sb[:], op=mybir.AluOpType.min)
        nc.vector.tensor_tensor(upd_sb[:], lo_sb[:], upd_sb[:], op=mybir.AluOpType.max)
        nc.sync.dma_start(out_d.ap(), upd_sb[:])

    nc.compile()
    _PROGRAM_CACHE[key] = nc
    return nc


def kernel(preds, pos_head, neg_head, pos_body, neg_body, atoms):
    global _LAST_RESULTS
    preds = np.ascontiguousarray(np.asarray(preds, dtype=np.float32))
    pos_head = np.asarray(pos_head)
    neg_head = np.asarray(neg_head)
    pos_body = np.asarray(pos_body)
    neg_body = np.asarray(neg_body)
    atoms_np = np.asarray(atoms).astype(np.int64)

    m = np.ascontiguousarray(preds[:, atoms_np].astype(np.float32))  # [B, N]
    # m_ext columns: [0..N) = m, N = 1.0 (pos pad), N+1 = 0.0 (neg/dummy pad)
    m_ext = np.concatenate(
        [m, np.ones((B, 1), np.float32), np.zeros((B, 1), np.float32)], axis=1
    )
    POS_PAD, NEG_PAD = N, N + 1

    pb = pos_body != 0
    nb_ = neg_body != 0
    kp_c = pb.sum(1)
    kn_c = nb_.sum(1)
    kph = max(_roundup(int(kp_c.max()), 4), 4)
    knh = max(_roundup(int(kn_c.max()), 4), 4)

    body_js = [
        (np.nonzero(pb[c])[0], np.nonzero(nb_[c])[0]) for c in range(C)
    ]

    # Head occurrences: one slot per (constraint, sign) head.
    ph_atom = pos_head.argmax(1)
    ph_has = pos_head.max(1) > 0
    nh_atom = neg_head.argmax(1)
    nh_has = neg_head.max(1) > 0
    pos_bins = [[] for _ in range(N)]
    neg_bins = [[] for _ in range(N)]
    for c in np.nonzero(ph_has)[0]:
        pos_bins[ph_atom[c]].append(c)
    for c in np.nonzero(nh_has)[0]:
        neg_bins[nh_atom[c]].append(c)

    # Per-atom max body widths over its bins' constraints.
    atom_kp = np.zeros(N, np.int64)
    atom_kn = np.zeros(N, np.int64)
    for n in range(N):
        cs = pos_bins[n] + neg_bins[n]
        if cs:
            atom_kp[n] = max(kp_c[c] for c in cs)
            atom_kn[n] = max(kn_c[c] for c in cs)

    # Pick the light-tier thresholds minimizing total packed slot bytes.
    best = None
    for kpl in (8, 12, 16, 20, kph):
        for knl in (8, 12, 16, 20, 24, knh):
            light = (atom_kp <= kpl) & (atom_kn <= knl)
            nslots = np.array([len(pos_bins[n]) + len(neg_bins[n]) for n in range(N)])
            cost = (nslots[light].sum() * (kpl + knl)
                    + nslots[~light].sum() * (kph + knh))
            if best is None or cost < best[0]:
                best = (cost, kpl, knl)
    _, kpl, knl = best
    wl, wh = kpl + knl, kph + knh
    atom_heavy = (atom_kp > kpl) | (atom_kn > knl)

    # Group atoms by (heavy, sp, sn); deal round-robin to the 8 cores.
    from collections import defaultdict

    group_atoms = defaultdict(list)
    for n in range(N):
        group_atoms[(bool(atom_heavy[n]), len(pos_bins[n]), len(neg_bins[n]))].append(n)

    # Light groups first: slot index space is [light slots][heavy slots].
    gkeys = sorted(group_atoms)  # False < True
    n_light_slots = sum(
        -(-len(group_atoms[k]) // NCORES) * (k[1] + k[2]) for k in gkeys if not k[0]
    )
    sl_pad = _roundup(max(n_light_slots, N_LIGHT_CHUNKS), N_LIGHT_CHUNKS)

    groups = []  # (sp, sn, cnt, col_off, slot_off) in combined slot space
    core_atoms = [[] for _ in range(NCORES)]  # (group_idx, pos_in_group, atom)
    col_off = 0
    slot_l = 0
    slot_h = sl_pad
    for key in gkeys:
        heavy, sp, sn = key
        atoms_g = group_atoms[key]
        cnt = -(-len(atoms_g) // NCORES)
        for j, a in enumerate(atoms_g):
            core_atoms[j % NCORES].append((len(groups), j // NCORES, a))
        soff = slot_h if heavy else slot_l
        groups.append((sp, sn, cnt, col_off, soff))
        col_off += cnt
        if heavy:
            slot_h += cnt * (sp + sn)
        else:
            slot_l += cnt * (sp + sn)
    assert slot_l <= sl_pad
    sh_pad = _roundup(slot_h - sl_pad, 2)
    nl_pad = _roundup(col_off, 4)

    dims = (kpl, knl, kph, knh, sl_pad, sh_pad, nl_pad)
    nc = _build_program(dims, tuple(groups))

    in_maps = []
    out_cols = []  # per core: (cols, atom_ids) to scatter back
    for core in range(NCORES):
        light_rows = np.full((sl_pad, wl), NEG_PAD, np.int32)
        heavy_rows = np.full((max(sh_pad, 1), wh), NEG_PAD, np.int32)
        mloc_idx = np.full(nl_pad, NEG_PAD, np.int32)  # dummy -> 0.0
        cols = []
        atom_ids = []
        for gi, pos_in_g, a in core_atoms[core]:
            sp, sn, cnt, coff, soff = groups[gi]
            heavy = soff >= sl_pad
            rows, kp_w, base0 = (
                (heavy_rows, kph, soff - sl_pad) if heavy else (light_rows, kpl, soff)
            )
            base = base0 + pos_in_g * (sp + sn)
            for l, cid in enumerate(pos_bins[a] + neg_bins[a]):
                jp, jn = body_js[cid]
                rows[base + l, : jp.size] = jp
                rows[base + l, jp.size : kp_w] = POS_PAD
                rows[base + l, kp_w : kp_w + jn.size] = jn
            mloc_idx[coff + pos_in_g] = a
            cols.append(coff + pos_in_g)
            atom_ids.append(a)
        gl_vals = m_ext[:, light_rows.ravel()]
        gh_vals = m_ext[:, heavy_rows.ravel()]
        chunks = _chunk_plan(kpl, knl, kph, knh, sl_pad, sh_pad)
        im = {}
        for i, (reg, s0, s1) in enumerate(chunks):
            vals, w = (gl_vals, wl) if reg == "l" else (gh_vals, wh)
            im[f"c{i}"] = np.ascontiguousarray(vals[:, s0 * w : s1 * w])
        im["mloc"] = np.ascontiguousarray(m_ext[:, mloc_idx])
        in_maps.append(im)
        out_cols.append((np.array(cols), np.array(atom_ids)))

    res = run_bass_kernel_spmd(
        nc, in_maps, core_ids=list(range(NCORES)), trace=_TRACE
    )
    _LAST_RESULTS = res

    out = preds.copy()
    for core in range(NCORES):
        cols, atom_ids = out_cols[core]
        if len(cols):
            out[:, atoms_np[atom_ids]] = res.results[core]["upd"][:, cols]
    return out


# revision 22
# speedup vs baseline: 1.2612x; 1.2612x over previous
"""Trainium2 Bass kernel for nn_ConstraintsModule.

Reference math:
    m = preds[:, atoms]                                   # [B, N]
    body_rev[b,c,j] = pos_body[c,j] + m[b,j]*(neg_body-pos_body)[c,j]
    body_min[b,c]   = 1 - max_j body_rev[b,c,j]
    lb[b,n] = max_c body_min[b,c]*pos_head[c,n]
    ub[b,n] = 1 - max_c body_min[b,c]*neg_head[c,n]
    updated = clamp(m, min(lb,ub), max(lb,ub))
    out = preds with columns `atoms` replaced by updated

Structure exploited:
  * body masks are ~2% dense: max_j body_rev collapses to
    max(1 - min_{j in pos} m, max_{j in neg} m) over ~20 literals.
  * head rows are one-hot: lb/ub are segment maxes of body_min grouped by
    (head atom, sign).

Layout: all 128 batch rows on the SBUF partition axis.  The host packs,
per constraint slot, pos-literal m values (padded with 1.0) and
neg-literal m values (padded with 0.0); slots are grouped into a "light"
region (small uniform width) and a "heavy" region (full width) to cut
padding bytes.  Device work is pure DVE: strided tensor_reduce min/max
per region chunk (overlapped with the chunk DMAs), an exact
body_min = 1-max(1-minP, maxQ) rewrite, per-(atom-group) segment maxes,
and the final clamp.  Every op rounds exactly like the reference, so the
result is bit-identical to the fp32 reference.

Sharding: atoms are grouped by (heavy, pos-bin-size, neg-bin-size) and
dealt round-robin to the 8 cores, so all cores share one SPMD program
(groups padded to the cross-core max count); only packed data differs.
"""

import sys
from contextlib import ExitStack

import numpy as np

if "/opt/trn_rl_repo" not in sys.path:
    sys.path.insert(0, "/opt/trn_rl_repo")

import concourse.bacc as bacc
import concourse.tile as tile
from concourse import mybir
from concourse.bass_utils import run_bass_kernel_spmd

B = 128
C = 1024
N = 512
NCORES = 8
N_LIGHT_CHUNKS = 5

# Set by test.py to profile; the grading path leaves these alone.
_TRACE = False
_LAST_RESULTS = None

_PROGRAM_CACHE: dict = {}


def _roundup(x: int, mult: int) -> int:
    return ((x + mult - 1) // mult) * mult


def _chunk_plan(kpl, knl, kph, knh, sl_pad, sh_pad):
    """Graded chunks (small first, so the first DVE reduce can start as
    early as possible) over [light slots][heavy slots], alternating between
    the two fast HWDGE queues (scalar, gpsimd)."""
    wl, wh = kpl + knl, kph + knh
    work = [("l", sl_pad, wl)]
    if sh_pad:
        work.append(("h", sh_pad, wh))
    total = sl_pad * wl + sh_pad * wh
    # Graded: small first (early DVE start), small last (short post-DMA tail).
    fracs = [0.07, 0.09, 0.13, 0.18, 0.24, 0.21, 0.08]
    bounds = []
    acc = 0.0
    for f in fracs[:-1]:
        acc += f
        bounds.append(int(total * acc))
    chunks = []  # (region, s0, s1)
    done = 0
    for reg, nslots, w in work:
        s = 0
        while s < nslots:
            nxt = [b for b in bounds if b > done]
            budget = (nxt[0] - done) if nxt else (total - done)
            take = min(max(budget // w, 1), nslots - s)
            chunks.append((reg, s, s + take))
            s += take
            done += take * w
    return tuple(chunks)


def _build_program(dims, groups):
    """dims = (kpl, knl, kph, knh, sl_pad, sh_pad, nl_pad);
    groups: tuple of (sp, sn, cnt, col_off, slot_off) in the combined slot
    space (light slots first, then heavy)."""
    key = (dims, groups)
    if key in _PROGRAM_CACHE:
        return _PROGRAM_CACHE[key]
    kpl, knl, kph, knh, sl_pad, sh_pad, nl_pad = dims

    dt = mybir.dt
    wl, wh = kpl + knl, kph + knh
    s_tot = sl_pad + sh_pad
    chunks = _chunk_plan(kpl, knl, kph, knh, sl_pad, sh_pad)

    nc = bacc.Bacc(
        "TRN2", target_bir_lowering=False, debug=False, enable_partition_id=False
    )
    c_ds = [
        nc.dram_tensor(
            f"c{i}", [B, (s1 - s0) * (wl if reg == "l" else wh)], dt.float32,
            kind="ExternalInput",
        )
        for i, (reg, s0, s1) in enumerate(chunks)
    ]
    mloc_d = nc.dram_tensor("mloc", [B, nl_pad], dt.float32, kind="ExternalInput")
    out_d = nc.dram_tensor("upd", [B, nl_pad], dt.float32, kind="ExternalOutput")

    with ExitStack() as ctx:
        tc = ctx.enter_context(tile.TileContext(nc))
        pool = ctx.enter_context(tc.tile_pool(name="main", bufs=1))

        mloc_sb = pool.tile([B, nl_pad], dt.float32, tag="mloc")
        nc.sync.dma_start(mloc_sb[:], mloc_d.ap())

        gl_sb = pool.tile([B, sl_pad * wl], dt.float32, tag="gl")
        gh_sb = pool.tile([B, max(sh_pad, 1) * wh], dt.float32, tag="gh")
        minp_sb = pool.tile([B, s_tot], dt.float32, tag="minp")
        maxq_sb = pool.tile([B, s_tot], dt.float32, tag="maxq")
        # G rides only the two fast HWDGE queues; sync carries mloc/out.
        dma_engines = [nc.scalar, nc.gpsimd]
        for i, (reg, s0, s1) in enumerate(chunks):
            w, kp_w, g_t, base = (
                (wl, kpl, gl_sb, 0) if reg == "l" else (wh, kph, gh_sb, sl_pad)
            )
            dma_engines[i % 2].dma_start(g_t[:, s0 * w : s1 * w], c_ds[i].ap())
            g3 = g_t[:, s0 * w : s1 * w].rearrange("p (c k) -> p c k", k=w)
            nc.vector.tensor_reduce(
                minp_sb[:, base + s0 : base + s1], g3[:, :, 0:kp_w],
                axis=mybir.AxisListType.X, op=mybir.AluOpType.min,
            )
            nc.vector.tensor_reduce(
                maxq_sb[:, base + s0 : base + s1], g3[:, :, kp_w:w],
                axis=mybir.AxisListType.X, op=mybir.AluOpType.max,
            )

        # body_min = 1 - max(1 - minP, maxQ), rounded exactly like the
        # reference (which materializes each 1-m and 1-body_max).
        bmin_sb = pool.tile([B, s_tot], dt.float32, tag="bmin")
        nc.vector.tensor_scalar(
            minp_sb[:], minp_sb[:], -1.0, 1.0,
            op0=mybir.AluOpType.mult, op1=mybir.AluOpType.add,
        )
        nc.vector.tensor_tensor(
            minp_sb[:], minp_sb[:], maxq_sb[:], op=mybir.AluOpType.max
        )
        nc.vector.tensor_scalar(
            bmin_sb[:], minp_sb[:], -1.0, 1.0,
            op0=mybir.AluOpType.mult, op1=mybir.AluOpType.add,
        )

        # Head phase: segment maxes over (atom, sign) bins.
        lb_sb = pool.tile([B, nl_pad], dt.float32, tag="lb")
        ubm_sb = pool.tile([B, nl_pad], dt.float32, tag="ubm")
        nc.vector.memset(lb_sb[:], 0.0)
        nc.vector.memset(ubm_sb[:], 0.0)
        for sp, sn, cnt, col_off, slot_off in groups:
            w = sp + sn
            if w == 0:
                continue  # lb/ubm stay 0 from the memset
            seg = bmin_sb[:, slot_off : slot_off + cnt * w].rearrange(
                "p (a l) -> p a l", l=w
            )
            if sp > 0:
                nc.vector.tensor_reduce(
                    lb_sb[:, col_off : col_off + cnt], seg[:, :, 0:sp],
                    axis=mybir.AxisListType.X, op=mybir.AluOpType.max,
                )
            if sn > 0:
                nc.vector.tensor_reduce(
                    ubm_sb[:, col_off : col_off + cnt], seg[:, :, sp:w],
                    axis=mybir.AxisListType.X, op=mybir.AluOpType.max,
                )

        # updated = max(min(lb, ub), min(max(lb, ub), m)),  ub = 1 - ubm
        ub_sb = pool.tile([B, nl_pad], dt.float32, tag="ub")
        nc.vector.tensor_scalar(
            ub_sb[:], ubm_sb[:], -1.0, 1.0,
            op0=mybir.AluOpType.mult, op1=mybir.AluOpType.add,
        )
        lo_sb = pool.tile([B, nl_pad], dt.float32, tag="lo")
        nc.vector.tensor_tensor(lo_sb[:], lb_sb[:], ub_sb[:], op=mybir.AluOpType.min)
        hi_sb = pool.tile([B, nl_pad], dt.float32, tag="hi")
        nc.vector.tensor_tensor(hi_sb[:], lb_sb[:], ub_sb[:], op=mybir.AluOpType.max)
        upd_sb = pool.tile([B, nl_pad], dt.float32, tag="upd")
        nc.vector.tensor_tensor(upd_sb[:], hi_sb[:], mloc_sb[:], op=mybir.AluOpType.min)
        nc.vector.tensor_tensor(upd_sb[:], lo_sb[:], upd_sb[:], op=mybir.AluOpType.max)
        nc.sync.dma_start(out_d.ap(), upd_sb[:])

    nc.compile()
    _PROGRAM_CACHE[key] = nc
    return nc


def kernel(preds, pos_head, neg_head, pos_body, neg_body, atoms):
    global _LAST_RESULTS
    preds = np.ascontiguousarray(np.asarray(preds, dtype=np.float32))
    pos_head = np.asarray(pos_head)
    neg_head = np.asarray(neg_head)
    pos_body = np.asarray(pos_body)
    neg_body = np.asarray(neg_body)
    atoms_np = np.asarray(atoms).astype(np.int64)

    m = np.ascontiguousarray(preds[:, atoms_np].astype(np.float32))  # [B, N]
    # m_ext columns: [0..N) = m, N = 1.0 (pos pad), N+1 = 0.0 (neg/dummy pad)
    m_ext = np.concatenate(
        [m, np.ones((B, 1), np.float32), np.zeros((B, 1), np.float32)], axis=1
    )
    POS_PAD, NEG_PAD = N, N + 1

    pb = pos_body != 0
    nb_ = neg_body != 0
    kp_c = pb.sum(1)
    kn_c = nb_.sum(1)
    kph = max(_roundup(int(kp_c.max()), 4), 4)
    knh = max(_roundup(int(kn_c.max()), 4), 4)

    body_js = [
        (np.nonzero(pb[c])[0], np.nonzero(nb_[c])[0]) for c in range(C)
    ]

    # Head occurrences: one slot per (constraint, sign) head.
    ph_atom = pos_head.argmax(1)
    ph_has = pos_head.max(1) > 0
    nh_atom = neg_head.argmax(1)
    nh_has = neg_head.max(1) > 0
    pos_bins = [[] for _ in range(N)]
    neg_bins = [[] for _ in range(N)]
    for c in np.nonzero(ph_has)[0]:
        pos_bins[ph_atom[c]].append(c)
    for c in np.nonzero(nh_has)[0]:
        neg_bins[nh_atom[c]].append(c)

    # Per-atom max body widths over its bins' constraints.
    atom_kp = np.zeros(N, np.int64)
    atom_kn = np.zeros(N, np.int64)
    for n in range(N):
        cs = pos_bins[n] + neg_bins[n]
        if cs:
            atom_kp[n] = max(kp_c[c] for c in cs)
            atom_kn[n] = max(kn_c[c] for c in cs)

    # Pick light-tier thresholds + bin-size bucketing minimizing the true
    # per-core packed bytes (cross-core ceil padding included).  Bucketing
    # bins up to a multiple of bb adds dummy all-zero constraint slots
    # (bmin=0, neutral in the bin max) but merges groups, cutting both the
    # ceil padding and the head-phase instruction count.
    from collections import Counter, defaultdict

    nsp = np.array([len(pos_bins[n]) for n in range(N)])
    nsn = np.array([len(neg_bins[n]) for n in range(N)])

    def structure(kpl_, knl_, bb):
        heavy = (atom_kp > kpl_) | (atom_kn > knl_)
        cnt = Counter()
        keys = []
        for n in range(N):
            spb = -(-int(nsp[n]) // bb) * bb if nsp[n] else 0
            snb = -(-int(nsn[n]) // bb) * bb if nsn[n] else 0
            key = (bool(heavy[n]), spb, snb)
            keys.append(key)
            cnt[key] += 1
        cost = sum(
            -(-c // NCORES) * (kk[1] + kk[2]) * ((kph + knh) if kk[0] else (kpl_ + knl_))
            for kk, c in cnt.items()
        )
        return cost, cnt, keys

    best = None
    for kpl_c in (8, 12, 16, 20, kph):
        for knl_c in (8, 12, 16, 20, 24, knh):
            for bb in (1, 2, 4):
                cost, cnt, keys = structure(kpl_c, knl_c, bb)
                rank = (cost, len(cnt) * 8)
                if best is None or rank < best[0]:
                    best = (rank, kpl_c, knl_c, bb, keys)
    _, kpl, knl, bb, atom_keys = best
    wl, wh = kpl + knl, kph + knh

    group_atoms = defaultdict(list)
    for n in range(N):
        group_atoms[atom_keys[n]].append(n)

    # Light groups first: slot index space is [light slots][heavy slots].
    gkeys = sorted(group_atoms)  # False < True
    n_light_slots = sum(
        -(-len(group_atoms[k]) // NCORES) * (k[1] + k[2]) for k in gkeys if not k[0]
    )
    sl_pad = _roundup(max(n_light_slots, N_LIGHT_CHUNKS), N_LIGHT_CHUNKS)

    groups = []  # (sp, sn, cnt, col_off, slot_off) in combined slot space
    core_atoms = [[] for _ in range(NCORES)]  # (group_idx, pos_in_group, atom)
    col_off = 0
    slot_l = 0
    slot_h = sl_pad
    for key in gkeys:
        heavy, sp, sn = key
        atoms_g = group_atoms[key]
        cnt = -(-len(atoms_g) // NCORES)
        for j, a in enumerate(atoms_g):
            core_atoms[j % NCORES].append((len(groups), j // NCORES, a))
        soff = slot_h if heavy else slot_l
        groups.append((sp, sn, cnt, col_off, soff))
        col_off += cnt
        if heavy:
            slot_h += cnt * (sp + sn)
        else:
            slot_l += cnt * (sp + sn)
    assert slot_l <= sl_pad
    sh_pad = _roundup(slot_h - sl_pad, 2)
    nl_pad = _roundup(col_off, 4)

    dims = (kpl, knl, kph, knh, sl_pad, sh_pad, nl_pad)
    nc = _build_program(dims, tuple(groups))

    in_maps = []
    out_cols = []  # per core: (cols, atom_ids) to scatter back
    for core in range(NCORES):
        light_rows = np.full((sl_pad, wl), NEG_PAD, np.int32)
        heavy_rows = np.full((max(sh_pad, 1), wh), NEG_PAD, np.int32)
        mloc_idx = np.full(nl_pad, NEG_PAD, np.int32)  # dummy -> 0.0
        cols = []
        atom_ids = []
        for gi, pos_in_g, a in core_atoms[core]:
            sp, sn, cnt, coff, soff = groups[gi]
            heavy = soff >= sl_pad
            rows, kp_w, base0 = (
                (heavy_rows, kph, soff - sl_pad) if heavy else (light_rows, kpl, soff)
            )
            base = base0 + pos_in_g * (sp + sn)
            for l, cid in enumerate(pos_bins[a]):
                jp, jn = body_js[cid]
                rows[base + l, : jp.size] = jp
                rows[base + l, jp.size : kp_w] = POS_PAD
                rows[base + l, kp_w : kp_w + jn.size] = jn
            for l, cid in enumerate(neg_bins[a]):
                jp, jn = body_js[cid]
                rows[base + sp + l, : jp.size] = jp
                rows[base + sp + l, jp.size : kp_w] = POS_PAD
                rows[base + sp + l, kp_w : kp_w + jn.size] = jn
            mloc_idx[coff + pos_in_g] = a
            cols.append(coff + pos_in_g)
            atom_ids.append(a)
        gl_vals = m_ext[:, light_rows.ravel()]
        gh_vals = m_ext[:, heavy_rows.ravel()]
        chunks = _chunk_plan(kpl, knl, kph, knh, sl_pad, sh_pad)
        im = {}
        for i, (reg, s0, s1) in enumerate(chunks):
            vals, w = (gl_vals, wl) if reg == "l" else (gh_vals, wh)
            im[f"c{i}"] = np.ascontiguousarray(vals[:, s0 * w : s1 * w])
        im["mloc"] = np.ascontiguousarray(m_ext[:, mloc_idx])
        in_maps.append(im)
        out_cols.append((np.array(cols), np.array(atom_ids)))

    res = run_bass_kernel_spmd(
        nc, in_maps, core_ids=list(range(NCORES)), trace=_TRACE
    )
    _LAST_RESULTS = res

    out = preds.copy()
    for core in range(NCORES):
        cols, atom_ids = out_cols[core]
        if len(cols):
            out[:, atoms_np[atom_ids]] = res.results[core]["upd"][:, cols]
    return out
